# revision 1
# baseline (speedup 1.0000x reference)
"""DFlashAttention Trainium2 kernel (8 NeuronCores).

Sharding: batch (2) data-parallel x kv-head-group (4) tensor-parallel = 8 cores.
Core c handles batch b=c//4, kv head g=c%4, q heads [4g..4g+4).
Host pre-transposes all operands so every on-device matmul contraction dim is
already the partition dim; o_proj partials are summed on host (the all-reduce).

Device pipeline per core (software-pipelined over 9 kv blocks of 512):
  QT = WqT.T @ XdT            -> [hd*4, 512] per-head RMSNorm + RoPE (PE-bcast trick)
  per 512-wide kv block:
    KT/VT = W{k,v}T.T @ XkvT  -> [128, 512];  K: RMSNorm + RoPE;  V: PE-transpose
    ST[c,q] = KT_tile.T @ QT  -> exp on ACT (no max subtraction; scores bounded ~5.3)
    pacc[h] += P              (softmax denominators accumulated on GPSIMD)
    OT[hd,q] += V_tile.T @ P  (flash accumulation in PSUM, unnormalized)
  denom = ones.T @ pacc;  OT /= denom (PE broadcast of reciprocal)
  out = OT.T @ WoT (partial; host sums the 4 cores of each batch = all-reduce)

dtypes: bf16 operands for all PE matmuls except the RoPE rotate-half permutation,
V-transposes and o_proj-normalize helpers (fp32r; bf16 corrupts those on HW),
fp32 PSUM accumulation throughout, fp32 output.
"""

import numpy as np
import ml_dtypes

import concourse.bass as bass
import concourse.mybir as mybir
from concourse import bacc
from concourse.tile import TileContext
from concourse import bass_utils

F32 = mybir.dt.float32
F32R = mybir.dt.float32r
BF16 = mybir.dt.bfloat16

B, CTX, DRAFT, D = 2, 4096, 512, 2048
H, KVH, HD = 16, 4, 128
NH = H // KVH            # 4 q heads per core
TOT = CTX + DRAFT        # 4608
BLK = 512
NB = TOT // BLK          # 9 kv blocks
SQ = DRAFT               # 512 queries
EPS = 1e-6
THETA = 10000.0
SCALE = 1.0 / float(np.sqrt(HD))

_CACHE: dict = {}


def _build_nc(repeat: int = 1, pend_depth: int = 4, pex_bufs: int = 8, qk_bf16: bool = True,
              st_bufs: int = 3, ot_bufs: int = 1, pool_elem: bool = True, x4_bufs: int = 8):
    nc = bacc.Bacc()

    xd = nc.dram_tensor("xd", [D, SQ], BF16, kind="ExternalInput")
    xkv = nc.dram_tensor("xkv", [D, TOT], BF16, kind="ExternalInput")
    wq = nc.dram_tensor("wq", [D, NH * HD], BF16, kind="ExternalInput")
    wk = nc.dram_tensor("wk", [D, HD], BF16, kind="ExternalInput")
    wv = nc.dram_tensor("wv", [D, HD], BF16, kind="ExternalInput")
    wo = nc.dram_tensor("wo", [NH * HD, D], BF16, kind="ExternalInput")
    cosk_d = nc.dram_tensor("cosk", [HD, TOT], BF16, kind="ExternalInput")
    sink_d = nc.dram_tensor("sink", [HD, TOT], BF16, kind="ExternalInput")
    perm_d = nc.dram_tensor("perm", [HD, HD], F32R, kind="ExternalInput")
    ident_d = nc.dram_tensor("ident", [HD, HD], F32R, kind="ExternalInput")
    onesc_d = nc.dram_tensor("onesc", [HD, 1], F32R, kind="ExternalInput")
    onesr_d = nc.dram_tensor("onesr", [1, HD], F32R, kind="ExternalInput")
    wqn_d = nc.dram_tensor("wqn", [1, HD], F32R, kind="ExternalInput")
    wkn_d = nc.dram_tensor("wkn", [1, HD], F32R, kind="ExternalInput")
    out = nc.dram_tensor("out", [SQ, D], F32, kind="ExternalOutput")

    with nc.allow_low_precision("f32r rounding required by fp32r matmul consumers"), \
         TileContext(nc) as tc:
        with (
            tc.tile_pool(name="const", bufs=1) as cpool,
            tc.tile_pool(name="big", bufs=1) as bpool,
            tc.tile_pool(name="x4", bufs=x4_bufs) as x4pool,       # [128,4,512] streams
            tc.tile_pool(name="w4", bufs=4) as w4pool,       # woN (phase3, prefetched)
            tc.tile_pool(name="scr", bufs=2) as scr,         # norm/rope scratch
            tc.tile_pool(name="pex", bufs=pex_bufs) as pex,         # exp probs
            tc.tile_pool(name="acc", bufs=1) as accp,        # persistent sbuf accumulators
            tc.tile_pool(name="ps_proj", bufs=2, space="PSUM") as ps_proj,
            tc.tile_pool(name="ps_nrm", bufs=2, space="PSUM") as ps_nrm,
            tc.tile_pool(name="ps_st", bufs=st_bufs, space="PSUM") as ps_st,
            tc.tile_pool(name="ps_ot", bufs=ot_bufs, space="PSUM") as ps_ot,
        ):
            SDT = BF16 if qk_bf16 else F32R
            # persistent accumulators
            otsb = [accp.tile([HD, SQ], F32, name=f"otsb{h}") for h in range(NH)]
            pacc = [accp.tile([128, SQ], F32, name=f"pacc{h}") for h in range(NH)]
            qrope = [accp.tile([HD, SQ], SDT, name=f"qrope{h}") for h in range(NH)]

            def norm_rope(src_ps, wrow, cos_sb, sin_sb, csl, dst, tagpfx):
                """RMSNorm (per position, over partition dim) + RoPE on a
                [128, 512] tile in PSUM; writes f32r SBUF tile `dst`."""
                src_sb = scr.tile([128, BLK], F32, name=f"{tagpfx}_src", tag="srcsb")
                nc.vector.tensor_copy(src_sb[:, :], src_ps[:, :])
                sq = scr.tile([128, BLK], F32R, name=f"{tagpfx}_sq", tag="sq")
                if pool_elem:
                    nc.gpsimd.tensor_mul(sq[:, :], src_sb[:, :], src_sb[:, :])
                else:
                    nc.scalar.square(sq[:, :], src_sb[:, :])
                ssq = ps_nrm.tile([1, BLK], F32, name=f"{tagpfx}_ssq", tag="nrm")
                nc.tensor.matmul(ssq[:, :], onesc[:, :], sq[:, :], start=True, stop=True)
                srt = scr.tile([1, BLK], F32, name=f"{tagpfx}_srt", tag="rs")
                nc.scalar.activation(srt[:, :], ssq[:, :],
                                     mybir.ActivationFunctionType.Sqrt,
                                     bias=eps_t[:, :], scale=1.0 / HD)
                rs = scr.tile([1, BLK], F32R, name=f"{tagpfx}_rs", tag="rs")
                nc.vector.reciprocal(rs[:, :], srt[:, :])
                nf = ps_nrm.tile([128, BLK], F32, name=f"{tagpfx}_nf", tag="nrm")
                nc.tensor.matmul(nf[:, :], wrow[:, :], rs[:, :], start=True, stop=True)
                xn = scr.tile([128, BLK], F32R, name=f"{tagpfx}_xn", tag="xn")
                nc.vector.tensor_mul(xn[:, :], src_sb[:, :], nf[:, :])
                # rope: dst = xn*cos + (perm @ xn)*sin
                pr = ps_nrm.tile([128, BLK], F32, name=f"{tagpfx}_pr", tag="nrm")
                nc.tensor.matmul(pr[:, :], perm[:, :], xn[:, :], start=True, stop=True)
                t1 = scr.tile([128, BLK], F32, name=f"{tagpfx}_t1", tag="t1")
                if pool_elem:
                    nc.gpsimd.tensor_mul(t1[:, :], xn[:, :], cos_sb[:, csl])
                else:
                    nc.vector.tensor_mul(t1[:, :], xn[:, :], cos_sb[:, csl])
                t2 = scr.tile([128, BLK], F32, name=f"{tagpfx}_t2", tag="sq")
                nc.vector.tensor_mul(t2[:, :], pr[:, :], sin_sb[:, csl])
                nc.vector.tensor_add(dst[:, :], t1[:, :], t2[:, :])

            # ---- phase 1: Q projection DMAs + MMs ----
            xd4 = []
            wq4 = []
            for dg in range(4):
                xt = x4pool.tile([128, 4, BLK], BF16, name=f"xd4_{dg}", tag="x4")
                nc.sync.dma_start(
                    xt[:, :, :],
                    xd[dg * 512:(dg + 1) * 512, :].rearrange("(j p) c -> p j c", p=128))
                xd4.append(xt)
                wt = x4pool.tile([128, 4, BLK], BF16, name=f"wq4_{dg}", tag="x4")
                nc.sync.dma_start(
                    wt[:, :, :],
                    wq[dg * 512:(dg + 1) * 512, :].rearrange("(j p) c -> p j c", p=128))
                wq4.append(wt)
            # ---- constants / tables ----
            perm = cpool.tile([HD, HD], F32R, name="perm_sb")
            nc.sync.dma_start(perm[:, :], perm_d[:, :])
            ident = cpool.tile([HD, HD], F32R, name="ident_sb")
            nc.sync.dma_start(ident[:, :], ident_d[:, :])
            onesc = cpool.tile([HD, 1], F32R, name="onesc_sb")
            nc.sync.dma_start(onesc[:, :], onesc_d[:, :])
            onesr = cpool.tile([1, HD], F32R, name="onesr_sb")
            nc.sync.dma_start(onesr[:, :], onesr_d[:, :])
            wqn = cpool.tile([1, HD], F32R, name="wqn_sb")
            nc.sync.dma_start(wqn[:, :], wqn_d[:, :])
            wkn = cpool.tile([1, HD], F32R, name="wkn_sb")
            nc.sync.dma_start(wkn[:, :], wkn_d[:, :])
            eps_t = cpool.tile([1, 1], F32, name="eps_sb")
            nc.vector.memset(eps_t[:, :], EPS)
            onescb = cpool.tile([HD, 1], BF16, name="onescb_sb")
            nc.vector.memset(onescb[:, :], 1.0)
            wk_sb = bpool.tile([128, 16, HD], BF16, name="wk_sb")
            nc.sync.dma_start(wk_sb[:, :, :], wk[:, :].rearrange("(g p) h -> p g h", p=128))
            wv_sb = bpool.tile([128, 16, HD], BF16, name="wv_sb")
            nc.sync.dma_start(wv_sb[:, :, :], wv[:, :].rearrange("(g p) h -> p g h", p=128))

            SDT = BF16 if qk_bf16 else F32R
            psqs = []
            for h in range(NH):
                psq = ps_st.tile([HD, SQ], F32, name=f"psq{h}", tag="st")
                for dg in range(4):
                    for j in range(4):
                        nc.tensor.matmul(
                            psq[:, :],
                            wq4[dg][:, j, h * HD:(h + 1) * HD],
                            xd4[dg][:, j, :],
                            start=(dg == 0 and j == 0),
                            stop=(dg == 3 and j == 3))
                psqs.append(psq)

            # ---- phase 2: software-pipelined kv blocks ----
            # PE emission order per block b:
            #   [K/V proj MMs b] ... [norm-chain MMs b-1 + V transposes b-1]
            #   ... [scores/exp/denom/attn b-2, 2-ahead st emission]
            # so ACT/DVE chain latencies hide behind dense projection MMs.
            state: dict = {}
            consts: dict = {}
            pfx = [""]

            def load_block(cb):
                csl = slice(cb * BLK, (cb + 1) * BLK)
                xk4 = []
                for dg in range(4):
                    xt = x4pool.tile([128, 4, BLK], BF16, name=f"{pfx[0]}xk4_{cb}_{dg}", tag="x4")
                    nc.sync.dma_start(
                        xt[:, :, :],
                        xkv[dg * 512:(dg + 1) * 512, csl].rearrange("(j p) c -> p j c", p=128))
                    xk4.append(xt)
                state[("xk4", cb)] = xk4

            def proj_block(cb):
                xk4 = state.pop(("xk4", cb))
                kt_ps = ps_proj.tile([HD, BLK], F32, name=f"{pfx[0]}kt{cb}", tag="proj")
                for dg in range(4):
                    for j in range(4):
                        nc.tensor.matmul(kt_ps[:, :], wk_sb[:, dg * 4 + j, :],
                                         xk4[dg][:, j, :],
                                         start=(dg == 0 and j == 0),
                                         stop=(dg == 3 and j == 3))
                vt_ps = ps_proj.tile([HD, BLK], F32, name=f"{pfx[0]}vt{cb}", tag="proj")
                for dg in range(4):
                    for j in range(4):
                        nc.tensor.matmul(vt_ps[:, :], wv_sb[:, dg * 4 + j, :],
                                         xk4[dg][:, j, :],
                                         start=(dg == 0 and j == 0),
                                         stop=(dg == 3 and j == 3))
                state[("kt", cb)] = kt_ps
                state[("vt", cb)] = vt_ps

            def prep_block(cb):
                """norm+rope on K, transpose V — chain MMs for block cb."""
                cosk = consts["cosk"]
                sink = consts["sink"]
                csl = slice(cb * BLK, (cb + 1) * BLK)
                kt_ps = state.pop(("kt", cb))
                vt_ps = state.pop(("vt", cb))
                ktf = scr.tile([HD, BLK], SDT, name=f"{pfx[0]}ktf{cb}", tag="ktf", bufs=2)
                norm_rope(kt_ps, wkn, cosk, sink, csl, ktf, f"{pfx[0]}k{cb}")
                vt_sb = scr.tile([HD, BLK], F32R, name=f"{pfx[0]}vt_sb{cb}", tag="vtsb")
                nc.vector.tensor_copy(vt_sb[:, :], vt_ps[:, :])
                tr_ps = ps_proj.tile([128, BLK], F32R, name=f"{pfx[0]}tr{cb}", tag="proj")
                vnat = []
                for j in range(4):
                    nc.tensor.transpose(tr_ps[:, j * HD:(j + 1) * HD],
                                        vt_sb[:, j * HD:(j + 1) * HD], ident[:, :])
                for j in range(4):
                    vn = scr.tile([128, HD], SDT, name=f"{pfx[0]}vn{cb}_{j}", tag=f"vn{j}", bufs=1)
                    nc.vector.tensor_copy(vn[:, :], tr_ps[:, j * HD:(j + 1) * HD])
                    vnat.append(vn)
                state[("ktf", cb)] = ktf
                state[("vnat", cb)] = vnat

            otn = [None] * NH

            def normalize_head(h):
                dsq = scr.tile([128, SQ], F32R, name=f"dsq{h}", tag="sq")
                nc.gpsimd.tensor_copy(dsq[:, :], pacc[h][:, :])
                den_ps = ps_nrm.tile([1, SQ], F32, name=f"den{h}", tag="nrm")
                lastp = state.pop(("lastpe", h), [])
                nc.tensor.matmul(den_ps[:, :], onesc[:, :], dsq[:, :],
                                 start=True, stop=(len(lastp) == 0))
                for i, pe_t in enumerate(lastp):
                    nc.tensor.matmul(den_ps[:, :], onescb[:, :], pe_t[:, :],
                                     start=False, stop=(i == len(lastp) - 1))
                rdt = scr.tile([1, SQ], F32, name=f"rdt{h}", tag="rs")
                nc.vector.tensor_copy(rdt[:, :], den_ps[:, :])
                rd = scr.tile([1, SQ], F32R, name=f"rd{h}", tag="rs")
                nc.vector.reciprocal(rd[:, :], rdt[:, :])
                nf = ps_nrm.tile([128, SQ], F32, name=f"onf{h}", tag="nrm")
                nc.tensor.matmul(nf[:, :], onesr[:, :], rd[:, :], start=True, stop=True)
                ot = accp.tile([HD, SQ], BF16, name=f"otn{h}")
                nc.vector.tensor_mul(ot[:, :], otsb[h][:, :], nf[:, :])
                otn[h] = ot

            def attn_block(cb):
                ktf = state.pop(("ktf", cb))
                vnat = state.pop(("vnat", cb))
                # 2-ahead pipeline: st MMs run ahead of exp-dependent den/ot MMs
                pend = []

                def flush_one():
                    h, j, p_exp, ot_ps = pend.pop(0)
                    nc.tensor.matmul(ot_ps[:, :], vnat[j][:, :], p_exp[:, :],
                                     start=(j == 0), stop=(j == 3))
                    last = cb == NB - 1 and state.get("last_rep")
                    if last:
                        # last block: denominator goes through PE in normalize_head
                        state.setdefault(("lastpe", h), []).append(p_exp)
                    elif cb == 0 and j == 0:
                        nc.gpsimd.tensor_copy(pacc[h][:, :], p_exp[:, :])
                    else:
                        nc.gpsimd.tensor_add(pacc[h][:, :], pacc[h][:, :], p_exp[:, :])
                    if j == 3:
                        if cb == 0:
                            nc.vector.tensor_copy(otsb[h][:, :], ot_ps[:, :])
                        else:
                            nc.vector.tensor_add(otsb[h][:, :], otsb[h][:, :], ot_ps[:, :])
                        if last:
                            normalize_head(h)

                for h in range(NH):
                    ot_ps = ps_ot.tile([HD, SQ], F32, name=f"{pfx[0]}ot{cb}_{h}", tag="ot")
                    for j in range(4):
                        st_ps = ps_st.tile([128, SQ], F32, name=f"{pfx[0]}st{cb}_{h}_{j}", tag="st")
                        nc.tensor.matmul(st_ps[:, :], ktf[:, j * HD:(j + 1) * HD],
                                         qrope[h][:, :], start=True, stop=True)
                        p_exp = pex.tile([128, SQ], SDT, name=f"{pfx[0]}pe{cb}_{h}_{j}", tag="pex")
                        nc.scalar.activation(p_exp[:, :], st_ps[:, :],
                                             mybir.ActivationFunctionType.Exp,
                                             scale=SCALE)
                        pend.append((h, j, p_exp, ot_ps))
                        if len(pend) >= pend_depth:
                            flush_one()
                while pend:
                    flush_one()

            # pipeline schedule
            for rep in range(repeat):
                pfx[0] = f"r{rep}_" if repeat > 1 else ""
                state["last_rep"] = (rep == repeat - 1)
                if rep == 0:
                    cosk = bpool.tile([HD, TOT], BF16, name="cosk_sb")
                    nc.sync.dma_start(cosk[:, :], cosk_d[:, :])
                    sink = bpool.tile([HD, TOT], BF16, name="sink_sb")
                    nc.sync.dma_start(sink[:, :], sink_d[:, :])
                    consts["cosk"] = cosk
                    consts["sink"] = sink
                load_block(0)
                load_block(1)
                if rep == 0:
                    pass
                cosk = consts["cosk"]
                sink = consts["sink"]
                proj_block(0)
                if rep == 0:
                    # Q norm chains (ACT/DVE work started during projections)
                    for h in range(NH):
                        norm_rope(psqs[h], wqn, cosk, sink, slice(CTX, TOT), qrope[h], f"q{h}")
                proj_block(1)
                prep_block(0)
                for cb in range(NB):
                    if cb + 2 < NB:
                        load_block(cb + 2)
                    if rep == repeat - 1 and cb == NB - 2:
                        for n in range(4):
                            woN = w4pool.tile([128, 4, 512], BF16, name=f"woN{n}", tag="w4")
                            nc.sync.dma_start(
                                woN[:, :, :],
                                wo[:, n * 512:(n + 1) * 512].rearrange("(h p) c -> p h c", p=128))
                            consts[f"woN{n}"] = woN
                    attn_block(cb)
                    if cb + 1 < NB:
                        prep_block(cb + 1)
                    if cb + 2 < NB:
                        proj_block(cb + 2)

            # ---- phase 3: o_proj (otn produced inside the last attn block) ----
            osbm = [scr.tile([128, D], F32, name=f"osbm{m}", tag=f"osbm{m}", bufs=1)
                    for m in range(4)]
            for n in range(4):
                nsl = slice(n * 512, (n + 1) * 512)
                woN = consts[f"woN{n}"]
                for m in range(4):
                    po = ps_st.tile([128, 512], F32, name=f"po{n}_{m}", tag="st")
                    for h in range(NH):
                        nc.tensor.matmul(po[:, :],
                                         otn[h][:, m * HD:(m + 1) * HD],
                                         woN[:, h, :],
                                         start=(h == 0), stop=(h == 3))
                    nc.vector.tensor_copy(osbm[m][:, nsl], po[:, :])
            for m in range(4):
                nc.sync.dma_start(out[m * 128:(m + 1) * 128, :], osbm[m][:, :])
    nc.finalize()
    return nc


def get_nc(repeat: int = 1, **kw):
    key = ("nc", repeat, tuple(sorted(kw.items())))
    if key not in _CACHE:
        _CACHE[key] = _build_nc(repeat, **kw)
    return _CACHE[key]


def _host_tables():
    inv = 1.0 / (THETA ** (np.arange(0, HD, 2, dtype=np.float32) / np.float32(HD)))
    inv2 = np.concatenate([inv, inv]).astype(np.float32)  # [128]
    pm = np.zeros((HD, HD), np.float32)
    pm[np.arange(64) + 64, np.arange(64)] = -1.0
    pm[np.arange(64), np.arange(64) + 64] = 1.0
    ident = np.eye(HD, dtype=np.float32)
    onesc = np.ones((HD, 1), np.float32)
    onesr = np.ones((1, HD), np.float32)
    return inv2, pm, ident, onesc, onesr


def _make_in_maps(inputs):
    draft = np.ascontiguousarray(np.asarray(inputs["draft_hidden"], np.float32))
    ctx = np.ascontiguousarray(np.asarray(inputs["context_hidden"], np.float32))
    Wq = np.asarray(inputs["Wq"], np.float32)
    Wk = np.asarray(inputs["Wk"], np.float32)
    Wv = np.asarray(inputs["Wv"], np.float32)
    Wo = np.asarray(inputs["Wo"], np.float32)
    qnw = np.asarray(inputs["q_norm_w"], np.float32).reshape(1, HD)
    knw = np.asarray(inputs["k_norm_w"], np.float32).reshape(1, HD)
    cpos = np.asarray(inputs["context_position_ids"])
    dpos = np.asarray(inputs["draft_position_ids"])

    inv2, pm, ident, onesc, onesr = _host_tables()

    in_maps = []
    for c in range(8):
        b, g = c // 4, c % 4
        kvin = np.concatenate([ctx[b], draft[b]], axis=0)       # [4608, 2048]
        xkvT = np.ascontiguousarray(kvin.T)                      # [2048, 4608]
        xdT = np.ascontiguousarray(draft[b].T)                   # [2048, 512]
        wqT = np.ascontiguousarray(Wq[4 * g * HD:(4 * g + 4) * HD, :].T)  # [2048, 512]
        wkT = np.ascontiguousarray(Wk[g * HD:(g + 1) * HD, :].T)          # [2048, 128]
        wvT = np.ascontiguousarray(Wv[g * HD:(g + 1) * HD, :].T)
        woT = np.ascontiguousarray(Wo[:, 4 * g * HD:(4 * g + 4) * HD].T)  # [512, 2048]
        fpos = np.concatenate([cpos[b], dpos[b]]).astype(np.float32)      # [4608]
        angk = inv2[:, None] * fpos[None, :]
        bf = ml_dtypes.bfloat16
        in_maps.append({
            "xd": xdT.astype(bf), "xkv": xkvT.astype(bf), "wq": wqT.astype(bf),
            "wk": wkT.astype(bf), "wv": wvT.astype(bf), "wo": woT.astype(bf),
            "cosk": np.cos(angk).astype(bf),
            "sink": np.sin(angk).astype(bf),
            "perm": pm, "ident": ident, "onesc": onesc, "onesr": onesr,
            "wqn": qnw, "wkn": knw,
        })
    return in_maps


def kernel(**inputs):
    in_maps = _make_in_maps(inputs)
    nc = get_nc()
    res = bass_utils.run_bass_kernel_spmd(nc, in_maps, core_ids=list(range(8)))
    outs = [res.results[c]["out"] for c in range(8)]
    full = np.stack([
        outs[0] + outs[1] + outs[2] + outs[3],
        outs[4] + outs[5] + outs[6] + outs[7],
    ]).astype(np.float32)
    return full



# revision 15
# speedup vs baseline: 1.2093x; 1.2093x over previous
"""DFlashAttention Trainium2 kernel (8 NeuronCores).

Sharding: batch (2) data-parallel x kv-head-group (4) tensor-parallel = 8 cores.
Core c handles batch b=c//4, kv head g=c%4, q heads [4g..4g+4).
o_proj partials are summed on host (the all-reduce).

Key structure (per core, software-pipelined over 9 kv blocks of 512):
 - K/V/Q projections in fp8 DoubleRow, 3 chains per projection at a uniform
   64x PSUM scale: wA@xhi + wA@xlo + rA@xhi with xhi=fp8(x), xlo=fp8(x-xhi),
   wA=fp8(64w), rA=fp8(64w-wA).  ~bf16-exact, 2.67x fewer PE cycles.
 - K/V projected TALL ([kv,hd]): rope is then a free-dim shuffle with a
   host-signed sin table (Pool), V needs no transpose at all, and the
   K rms term is a Pool square + DVE free-axis reduce -> msk[kv,1].
 - RMSNorm folded into softmax: probs = exp(st * scaleAP[kv]) where
   scaleAP = exp(-0.5*ln(mean(k~^2)+eps) + ln(SCALE/64)) via ACT Ln+Exp
   (both live in one activation table -> no table reloads; Sqrt banned).
   Q-side norm multiplies into qhat (PE broadcast); k_norm_w folded into
   q_norm_w on the host (elementwise product, both apply per head-dim).
 - Attention TALL: ot[q,hd] += P_slice^T(stationary) @ V[kv,hd], and the
   softmax denominators are stationary-P matmuls with a [128,1] output
   (~1 PE cycle each) accumulated across all blocks in a single PSUM bank
   via a zero-matmul start/stop bracket.  onecol=64.0 makes den=64*sum(p),
   cancelling the 64x V scale in the final reciprocal.
 - o_proj in bf16 as before; outputs DMA'd straight from PSUM.

dtypes: fp8e4(e4m3) for projection operands, bf16 scores/probs/attn/o_proj,
fp32 PSUM accumulation throughout, fp32 output.
"""

import numpy as np
import ml_dtypes

import concourse.bass as bass
import concourse.mybir as mybir
from concourse import bacc
from concourse.tile import TileContext
from concourse import bass_utils

F32 = mybir.dt.float32
F32R = mybir.dt.float32r
BF16 = mybir.dt.bfloat16
FP8 = mybir.dt.float8e4
DR = mybir.MatmulPerfMode.DoubleRow
AL = mybir.AluOpType
AF = mybir.ActivationFunctionType

B, CTX, DRAFT, D = 2, 4096, 512, 2048
H, KVH, HD = 16, 4, 128
NH = H // KVH            # 4 q heads per core
TOT = CTX + DRAFT        # 4608
BLK = 512
NB = TOT // BLK          # 9 kv blocks
SQ = DRAFT               # 512 queries
EPS = 1e-6
THETA = 10000.0
SCALE = 1.0 / float(np.sqrt(HD))
WS = 64.0                # fp8 weight pre-scale

_CACHE: dict = {}


def _build_nc(pend_depth: int = 4, x4_bufs: int = 16, pex_bufs: int = 10):
    nc = bacc.Bacc()

    xdh = nc.dram_tensor("xdh", [D, SQ], FP8, kind="ExternalInput")
    xdl = nc.dram_tensor("xdl", [D, SQ], FP8, kind="ExternalInput")
    xkh = nc.dram_tensor("xkh", [D, TOT], FP8, kind="ExternalInput")
    xkl = nc.dram_tensor("xkl", [D, TOT], FP8, kind="ExternalInput")
    wqa_d = nc.dram_tensor("wqa", [D, NH * HD], FP8, kind="ExternalInput")
    wqr_d = nc.dram_tensor("wqr", [D, NH * HD], FP8, kind="ExternalInput")
    wka_d = nc.dram_tensor("wka", [D, HD], FP8, kind="ExternalInput")
    wkr_d = nc.dram_tensor("wkr", [D, HD], FP8, kind="ExternalInput")
    wva_d = nc.dram_tensor("wva", [D, HD], FP8, kind="ExternalInput")
    wvr_d = nc.dram_tensor("wvr", [D, HD], FP8, kind="ExternalInput")
    wo_d = nc.dram_tensor("wo", [NH * HD, D], BF16, kind="ExternalInput")
    coskT_d = nc.dram_tensor("coskT", [TOT, HD], BF16, kind="ExternalInput")
    sinkT_d = nc.dram_tensor("sinkT", [TOT, HD], BF16, kind="ExternalInput")
    cosq_d = nc.dram_tensor("cosq", [HD, SQ], BF16, kind="ExternalInput")
    sinq_d = nc.dram_tensor("sinq", [HD, SQ], BF16, kind="ExternalInput")
    perm_d = nc.dram_tensor("perm", [HD, HD], F32R, kind="ExternalInput")
    identb_d = nc.dram_tensor("identb", [HD, HD], BF16, kind="ExternalInput")
    identr_d = nc.dram_tensor("identr", [HD, HD], F32R, kind="ExternalInput")
    onesc_d = nc.dram_tensor("onesc", [HD, 1], F32R, kind="ExternalInput")
    wqnk_d = nc.dram_tensor("wqnk", [1, HD], F32R, kind="ExternalInput")
    out = nc.dram_tensor("out", [SQ, D], F32, kind="ExternalOutput")

    with nc.allow_low_precision("fp8/f32r matmul pipeline"), \
         TileContext(nc) as tc:
        with (
            tc.tile_pool(name="const", bufs=1) as cpool,
            tc.tile_pool(name="wts", bufs=1) as wpool,
            tc.tile_pool(name="x4", bufs=x4_bufs) as x4pool,
            tc.tile_pool(name="ck", bufs=2) as ckpool,
            tc.tile_pool(name="scr", bufs=2) as scr,
            tc.tile_pool(name="pex", bufs=pex_bufs) as pex,
            tc.tile_pool(name="acc", bufs=1) as accp,
            tc.tile_pool(name="ps_proj", bufs=2, space="PSUM") as ps_proj,
            tc.tile_pool(name="ps_st", bufs=2, space="PSUM") as ps_st,
            tc.tile_pool(name="ps_tr", bufs=1, space="PSUM") as ps_tr,
            tc.tile_pool(name="ps_ot", bufs=2, space="PSUM") as ps_ot,
            tc.tile_pool(name="ps_den", bufs=1, space="PSUM") as ps_den,
        ):
            # ---- constants ----
            perm = cpool.tile([HD, HD], F32R, name="perm_sb")
            nc.sync.dma_start(perm[:, :], perm_d[:, :])
            identb = cpool.tile([HD, HD], BF16, name="identb_sb")
            nc.sync.dma_start(identb[:, :], identb_d[:, :])
            identr = cpool.tile([HD, HD], F32R, name="identr_sb")
            nc.sync.dma_start(identr[:, :], identr_d[:, :])
            onesc = cpool.tile([HD, 1], F32R, name="onesc_sb")
            nc.sync.dma_start(onesc[:, :], onesc_d[:, :])
            wqnk = cpool.tile([1, HD], F32R, name="wqnk_sb")
            nc.sync.dma_start(wqnk[:, :], wqnk_d[:, :])
            cosq = cpool.tile([HD, SQ], BF16, name="cosq_sb")
            nc.sync.dma_start(cosq[:, :], cosq_d[:, :])
            sinq = cpool.tile([HD, SQ], BF16, name="sinq_sb")
            nc.sync.dma_start(sinq[:, :], sinq_d[:, :])
            eps_t = cpool.tile([128, 1], F32, name="eps_sb")
            nc.vector.memset(eps_t[:, :], EPS)
            zb = cpool.tile([128, 1], F32, name="zb_sb")
            nc.vector.memset(zb[:, :], 0.0)
            lnk_t = cpool.tile([128, 1], F32, name="lnk_sb")
            nc.vector.memset(lnk_t[:, :], float(np.log(SCALE / WS)))
            lnq_t = cpool.tile([1, 1], F32, name="lnq_sb")
            nc.vector.memset(lnq_t[:, :], float(np.log(1.0 / WS)))
            onecol64 = cpool.tile([128, 1], BF16, name="onecol64_sb")
            nc.vector.memset(onecol64[:, :], WS)
            zcolw = cpool.tile([128, HD], BF16, name="zcolw_sb")
            nc.vector.memset(zcolw[:, :], 0.0)
            zrow16 = cpool.tile([128, 16], BF16, name="zrow16_sb")
            nc.vector.memset(zrow16[:, :], 0.0)

            # ---- weights ----
            wqa = wpool.tile([128, 16, NH * HD], FP8, name="wqa_sb")
            nc.sync.dma_start(wqa[:, :, :],
                              wqa_d[:, :].rearrange("(j p) c -> p j c", p=128))
            wqr = wpool.tile([128, 16, NH * HD], FP8, name="wqr_sb")
            nc.sync.dma_start(wqr[:, :, :],
                              wqr_d[:, :].rearrange("(j p) c -> p j c", p=128))
            wk8 = {}
            for nm, dt_ in (("wka", wka_d), ("wkr", wkr_d),
                            ("wva", wva_d), ("wvr", wvr_d)):
                t = wpool.tile([128, 16, HD], FP8, name=f"{nm}_sb")
                nc.sync.dma_start(t[:, :, :],
                                  dt_[:, :].rearrange("(j p) h -> p j h", p=128))
                wk8[nm] = t

            # xd split tiles
            xdh_sb = wpool.tile([128, 16, SQ], FP8, name="xdh_sb")
            nc.sync.dma_start(xdh_sb[:, :, :],
                              xdh[:, :].rearrange("(j p) c -> p j c", p=128))
            xdl_sb = wpool.tile([128, 16, SQ], FP8, name="xdl_sb")
            nc.sync.dma_start(xdl_sb[:, :, :],
                              xdl[:, :].rearrange("(j p) c -> p j c", p=128))

            # persistent accumulators
            otsbT = [accp.tile([128, NH, HD], F32, name=f"otsbT{h}")
                     for h in range(NH)]
            qrope = [accp.tile([HD, SQ], BF16, name=f"qrope{h}") for h in range(NH)]

            # ---- phase 1: Q projection (fp8 DR, 3 chains) + norm/rope ----
            for h in range(NH):
                hs = slice(h * HD, (h + 1) * HD)
                psq = ps_st.tile([HD, SQ], F32, name=f"psq{h}", tag="st")
                first = True
                for wtile, xtile in ((wqa, xdh_sb), (wqa, xdl_sb), (wqr, xdh_sb)):
                    for t in range(8):
                        nc.tensor.matmul(psq[:, :],
                                         wtile[:, 2 * t:2 * t + 2, hs],
                                         xtile[:, 2 * t:2 * t + 2, :],
                                         start=first,
                                         stop=(wtile is wqr and t == 7),
                                         perf_mode=DR)
                        first = False
                # norm + rope chain (all ACT via Ln/Exp table)
                src = scr.tile([128, SQ], F32, name=f"qsrc{h}", tag="qsrc")
                nc.vector.tensor_copy(src[:, :], psq[:, :])
                sq = scr.tile([128, SQ], F32R, name=f"qsq{h}", tag="qsq")
                nc.gpsimd.tensor_mul(sq[:, :], src[:, :], src[:, :])
                ssq = ps_tr.tile([1, SQ], F32, name=f"qssq{h}", tag="tr")
                nc.tensor.matmul(ssq[:, :], onesc[:, :], sq[:, :],
                                 start=True, stop=True)
                lnm = scr.tile([1, SQ], F32, name=f"qln{h}", tag="qln")
                nc.scalar.activation(lnm[:, :], ssq[:, :], AF.Ln,
                                     bias=eps_t[0:1, :],
                                     scale=1.0 / (HD * WS * WS))
                rs = scr.tile([1, SQ], F32R, name=f"qrs{h}", tag="qln")
                nc.scalar.activation(rs[:, :], lnm[:, :], AF.Exp,
                                     bias=lnq_t[:, :], scale=-0.5)
                nf = ps_st.tile([128, SQ], F32, name=f"qnf{h}", tag="st")
                nc.tensor.matmul(nf[:, :], wqnk[:, :], rs[:, :],
                                 start=True, stop=True)
                xn = scr.tile([128, SQ], F32R, name=f"qxn{h}", tag="qxn")
                nc.vector.tensor_mul(xn[:, :], src[:, :], nf[:, :])
                pr = ps_st.tile([128, SQ], F32, name=f"qpr{h}", tag="st")
                nc.tensor.matmul(pr[:, :], perm[:, :], xn[:, :],
                                 start=True, stop=True)
                t1 = scr.tile([128, SQ], F32R, name=f"qt1{h}", tag="qsq")
                nc.gpsimd.tensor_mul(t1[:, :], xn[:, :], cosq[:, :])
                t2 = scr.tile([128, SQ], F32, name=f"qt2{h}", tag="qxn")
                nc.vector.tensor_mul(t2[:, :], pr[:, :], sinq[:, :])
                nc.gpsimd.tensor_add(qrope[h][:, :], t1[:, :], t2[:, :])

            # denominator accumulator bracket start (held across phase 2)
            den_ps = ps_den.tile([128, 16], F32, name="den_ps")
            nc.tensor.matmul(den_ps[:, :], zcolw[:, :], zrow16[:, :],
                             start=True, stop=False)

            # ---- phase 2: pipelined kv blocks ----
            state: dict = {}

            def load_block(cb):
                csl = slice(cb * BLK, (cb + 1) * BLK)
                xs = []
                for src_d, nm in ((xkh, "xh"), (xkl, "xl")):
                    dg_tiles = []
                    for dg in range(4):
                        t = x4pool.tile([128, 4, BLK], FP8,
                                        name=f"{nm}{cb}_{dg}", tag="x4")
                        nc.sync.dma_start(
                            t[:, :, :],
                            src_d[dg * 512:(dg + 1) * 512, csl]
                            .rearrange("(j p) c -> p j c", p=128))
                        dg_tiles.append(t)
                    xs.append(dg_tiles)
                state[("x", cb)] = xs
                for src_d, nm in ((coskT_d, "cosk"), (sinkT_d, "sink")):
                    t = ckpool.tile([128, 4, HD], BF16, name=f"{nm}{cb}", tag=nm)
                    nc.sync.dma_start(
                        t[:, :, :],
                        src_d[csl, :].rearrange("(j p) h -> p j h", p=128))
                    state[(nm, cb)] = t

            def proj_block(cb):
                """K/V fp8-DR tall projections: out [kv,hd] per 128-chunk."""
                xh4, xl4 = state[("x", cb)]
                for wa, wr, key in (("wka", "wkr", "kt"), ("wva", "wvr", "vt")):
                    ps = ps_proj.tile([128, 4, HD], F32, name=f"{key}{cb}",
                                      tag="proj")
                    for c in range(4):
                        cs = slice(c * 128, (c + 1) * 128)
                        first = True
                        for wtile, x4 in ((wk8[wa], xh4), (wk8[wa], xl4),
                                          (wk8[wr], xh4)):
                            for dg in range(4):
                                for u in range(2):
                                    nc.tensor.matmul(
                                        ps[:, c, :],
                                        x4[dg][:, 2 * u:2 * u + 2, cs],
                                        wtile[:, dg * 4 + 2 * u:dg * 4 + 2 * u + 2, :],
                                        start=first,
                                        stop=(wtile is wk8[wr] and dg == 3
                                              and u == 1),
                                        perf_mode=DR)
                                    first = False
                    state[(key, cb)] = ps
                state.pop(("x", cb))

            def prep_block(cb):
                """rope K (Pool, signed-sin shuffle), msk->rsq, transpose K,
                copy V; produces ktf [hd,kv] bf16, vnat [kv,hd] bf16, rsq."""
                ktT = state.pop(("kt", cb))
                vtT = state.pop(("vt", cb))
                cosk = state.pop(("cosk", cb))
                sink = state.pop(("sink", cb))
                kts = scr.tile([128, 4, HD], F32R, name=f"kts{cb}", tag="kts")
                nc.vector.tensor_copy(kts[:, :, :], ktT[:, :, :])
                t1 = scr.tile([128, 4, HD], F32R, name=f"t1_{cb}", tag="t1")
                nc.gpsimd.tensor_mul(t1[:, :, :], kts[:, :, :], cosk[:, :, :])
                roped = scr.tile([128, 4, HD], F32R, name=f"rop{cb}", tag="rop")
                nc.gpsimd.tensor_mul(roped[:, :, 0:64], kts[:, :, 64:128],
                                     sink[:, :, 0:64])
                nc.gpsimd.tensor_mul(roped[:, :, 64:128], kts[:, :, 0:64],
                                     sink[:, :, 64:128])
                nc.gpsimd.tensor_add(roped[:, :, :], roped[:, :, :], t1[:, :, :])
                sq = scr.tile([128, 4, HD], F32R, name=f"sqk{cb}", tag="t1")
                nc.gpsimd.tensor_mul(sq[:, :, :], roped[:, :, :], roped[:, :, :])
                msk = scr.tile([128, 4], F32, name=f"msk{cb}", tag="msk")
                nc.vector.tensor_reduce(msk[:, :], sq[:, :, :],
                                        axis=mybir.AxisListType.X, op=AL.add)
                lnm = scr.tile([128, 4], F32, name=f"lnk{cb}", tag="msk")
                nc.scalar.activation(lnm[:, :], msk[:, :], AF.Ln,
                                     bias=eps_t[:, :],
                                     scale=1.0 / (HD * WS * WS))
                rsq = scr.tile([128, 4], F32, name=f"rsq{cb}", tag="rsq")
                nc.scalar.activation(rsq[:, :], lnm[:, :], AF.Exp,
                                     bias=lnk_t[:, :], scale=-0.5)
                # transpose roped K -> [hd, kv] (f32r), then one copy to bf16
                tr = ps_tr.tile([128, 4, HD], F32R, name=f"ktr{cb}", tag="tr")
                for j in range(4):
                    nc.tensor.transpose(tr[:, j, :], roped[:, j, :],
                                        identr[:, :])
                ktf = scr.tile([128, 4, HD], BF16, name=f"ktf{cb}", tag="ktf")
                nc.vector.tensor_copy(ktf[:, :, :], tr[:, :, :])
                vnat = scr.tile([128, 4, HD], BF16, name=f"vnat{cb}", tag="vnat")
                nc.vector.tensor_copy(vnat[:, :, :], vtT[:, :, :])
                state[("ktf", cb)] = ktf
                state[("vnat", cb)] = vnat
                state[("rsq", cb)] = rsq

            def attn_block(cb):
                ktf = state.pop(("ktf", cb))
                vnat = state.pop(("vnat", cb))
                rsq = state.pop(("rsq", cb))
                pend = []

                def flush_one():
                    h, j, p_t, ot_ps = pend.pop(0)
                    for qc in range(4):
                        qs = slice(qc * 128, (qc + 1) * 128)
                        nc.tensor.matmul(ot_ps[:, qc, :], p_t[:, qs],
                                         vnat[:, j, :],
                                         start=(j == 0 and qc == 0),
                                         stop=(j == 3 and qc == 3))
                        nc.tensor.matmul(den_ps[:, 4 * h + qc:4 * h + qc + 1],
                                         p_t[:, qs], onecol64[:, :],
                                         start=False, stop=False)
                    if j == 3:
                        if cb == 0:
                            nc.vector.tensor_copy(otsbT[h][:, :, :],
                                                  ot_ps[:, :, :])
                        else:
                            nc.vector.tensor_add(otsbT[h][:, :, :],
                                                 otsbT[h][:, :, :],
                                                 ot_ps[:, :, :])

                for h in range(NH):
                    ot_ps = ps_ot.tile([128, 4, HD], F32, name=f"ot{cb}_{h}",
                                       tag="ot")
                    for j in range(4):
                        st_ps = ps_st.tile([128, SQ], F32,
                                           name=f"st{cb}_{h}_{j}", tag="st")
                        nc.tensor.matmul(st_ps[:, :], ktf[:, j, :],
                                         qrope[h][:, :], start=True, stop=True)
                        p_t = pex.tile([128, SQ], BF16, name=f"p{cb}_{h}_{j}",
                                       tag="pex")
                        nc.scalar.activation(p_t[:, :], st_ps[:, :], AF.Exp,
                                             bias=zb[:, :],
                                             scale=rsq[:, j:j + 1])
                        pend.append((h, j, p_t, ot_ps))
                        if len(pend) >= pend_depth:
                            flush_one()
                while pend:
                    flush_one()

            # pipeline
            load_block(0)
            load_block(1)
            proj_block(0)
            prep_block(0)
            for cb in range(NB):
                if cb + 2 < NB:
                    load_block(cb + 2)
                if cb + 1 < NB:
                    proj_block(cb + 1)
                attn_block(cb)
                if cb + 1 < NB:
                    prep_block(cb + 1)
                if cb == NB - 2:
                    woN = []
                    for n in range(4):
                        t = x4pool.tile([128, 4, 512], BF16, name=f"woN{n}",
                                        tag="x4")
                        nc.sync.dma_start(
                            t[:, :, :],
                            wo_d[:, n * 512:(n + 1) * 512]
                            .rearrange("(h p) c -> p h c", p=128))
                        woN.append(t)

            # ---- phase 3: normalize, transpose back, o_proj ----
            nc.tensor.matmul(den_ps[:, :], zcolw[:, :], zrow16[:, :],
                             start=False, stop=True)
            rd = accp.tile([128, 16], F32, name="rd_sb")
            nc.vector.reciprocal(rd[:, :], den_ps[:, :])
            otn = []
            for h in range(NH):
                otnT = accp.tile([128, NH, HD], BF16, name=f"otnT{h}")
                for qc in range(4):
                    nc.scalar.activation(otnT[:, qc, :], otsbT[h][:, qc, :],
                                         AF.Copy,
                                         scale=rd[:, 4 * h + qc:4 * h + qc + 1])
                trh = ps_tr.tile([128, 4, HD], BF16, name=f"otr{h}", tag="tr")
                for qc in range(4):
                    nc.tensor.transpose(trh[:, qc, :], otnT[:, qc, :],
                                        identb[:, :])
                o_h = accp.tile([128, NH, HD], BF16, name=f"otn{h}")
                nc.vector.tensor_copy(o_h[:, :, :], trh[:, :, :])
                otn.append(o_h)

            for n in range(4):
                for m in range(4):
                    po = ps_st.tile([128, 512], F32, name=f"po{n}_{m}", tag="st")
                    for h in range(NH):
                        nc.tensor.matmul(po[:, :], otn[h][:, m, :],
                                         woN[n][:, h, :],
                                         start=(h == 0), stop=(h == 3))
                    osb = scr.tile([128, 512], F32, name=f"osb{n}_{m}",
                                   tag="osb", bufs=3)
                    nc.scalar.activation(osb[:, :], po[:, :], AF.Copy)
                    nc.sync.dma_start(
                        out[m * 128:(m + 1) * 128, n * 512:(n + 1) * 512],
                        osb[:, :])
    # All activation funcs used (Exp, Ln, Copy) live in one table set
    # (natural_log_exp_and_others).  The default greedy pass picks a
    # different "first matching" set per function and thrashes 26 table
    # reloads (~33us on ACT); pin the single covering set instead.
    import types
    from concourse.hw_specs import get_activation_tables

    def _pin_act_table(self):
        tables = list(get_activation_tables(self.m.arch).items())
        idx = [i for i, (nm, fs) in enumerate(tables)
               if nm == "natural_log_exp_and_others"][0]
        funcs = tables[idx][1]
        for blk in self.main_func.blocks:
            pos = None
            for i, inst in enumerate(blk.instructions):
                if isinstance(inst, mybir.InstActivation):
                    assert inst.func in funcs, f"{inst.func} not in pinned set"
                    if pos is None:
                        pos = i
            if pos is None:
                continue
            atl = mybir.InstLoadActFuncSet(
                name=self.get_next_instruction_name(), ins=[], outs=[],
                act_func_set_id=idx)
            atl.engine = mybir.EngineType.Activation
            self.register_instruction(atl)
            blk.instructions.insert(pos, atl)

    nc.insert_act_table_loads = types.MethodType(_pin_act_table, nc)
    nc.finalize()
    return nc


def get_nc(**kw):
    key = ("nc", tuple(sorted(kw.items())))
    if key not in _CACHE:
        _CACHE[key] = _build_nc(**kw)
    return _CACHE[key]


def _host_tables():
    inv = 1.0 / (THETA ** (np.arange(0, HD, 2, dtype=np.float32) / np.float32(HD)))
    inv2 = np.concatenate([inv, inv]).astype(np.float32)  # [128]
    pm = np.zeros((HD, HD), np.float32)
    pm[np.arange(64) + 64, np.arange(64)] = -1.0
    pm[np.arange(64), np.arange(64) + 64] = 1.0
    ident = np.eye(HD, dtype=np.float32)
    onesc = np.ones((HD, 1), np.float32)
    return inv2, pm, ident, onesc


def _make_in_maps(inputs):
    F8 = ml_dtypes.float8_e4m3
    bf = ml_dtypes.bfloat16
    draft = np.ascontiguousarray(np.asarray(inputs["draft_hidden"], np.float32))
    ctx = np.ascontiguousarray(np.asarray(inputs["context_hidden"], np.float32))
    Wq = np.asarray(inputs["Wq"], np.float32)
    Wk = np.asarray(inputs["Wk"], np.float32)
    Wv = np.asarray(inputs["Wv"], np.float32)
    Wo = np.asarray(inputs["Wo"], np.float32)
    qnw = np.asarray(inputs["q_norm_w"], np.float32).reshape(HD)
    knw = np.asarray(inputs["k_norm_w"], np.float32).reshape(HD)
    cpos = np.asarray(inputs["context_position_ids"])
    dpos = np.asarray(inputs["draft_position_ids"])

    inv2, pm, ident, onesc = _host_tables()

    def split8(x):
        hi = x.astype(F8)
        lo = (x - hi.astype(np.float32)).astype(F8)
        return hi, lo

    def wsplit(w):
        a = (WS * w).astype(F8)
        r = (WS * w - a.astype(np.float32)).astype(F8)
        return a, r

    in_maps = []
    for c in range(8):
        b, g = c // 4, c % 4
        kvin = np.concatenate([ctx[b], draft[b]], axis=0)        # [4608, 2048]
        xkvT = np.ascontiguousarray(kvin.T)                      # [2048, 4608]
        xdT = np.ascontiguousarray(draft[b].T)                   # [2048, 512]
        xkh, xkl = split8(xkvT)
        xdh, xdl = split8(xdT)
        wqa, wqr = wsplit(np.ascontiguousarray(
            Wq[4 * g * HD:(4 * g + 4) * HD, :].T))               # [2048, 512]
        wka, wkr = wsplit(np.ascontiguousarray(Wk[g * HD:(g + 1) * HD, :].T))
        wva, wvr = wsplit(np.ascontiguousarray(Wv[g * HD:(g + 1) * HD, :].T))
        woT = np.ascontiguousarray(Wo[:, 4 * g * HD:(4 * g + 4) * HD].T)

        fpos = np.concatenate([cpos[b], dpos[b]]).astype(np.float32)  # [4608]
        angkT = fpos[:, None] * inv2[None, :]                     # [4608, 128]
        coskT = np.cos(angkT)
        sinkT = np.sin(angkT)
        sinkT[:, 0:64] = -sinkT[:, 0:64]                          # host-signed
        angq = inv2[:, None] * dpos[b].astype(np.float32)[None, :]  # [128, 512]

        in_maps.append({
            "xdh": xdh, "xdl": xdl, "xkh": xkh, "xkl": xkl,
            "wqa": wqa, "wqr": wqr, "wka": wka, "wkr": wkr,
            "wva": wva, "wvr": wvr,
            "wo": woT.astype(bf),
            "coskT": coskT.astype(bf), "sinkT": sinkT.astype(bf),
            "cosq": np.cos(angq).astype(bf), "sinq": np.sin(angq).astype(bf),
            "perm": pm, "identb": ident.astype(bf), "identr": ident,
            "onesc": onesc,
            "wqnk": (qnw * knw).reshape(1, HD),
        })
    return in_maps


def kernel(**inputs):
    in_maps = _make_in_maps(inputs)
    nc = get_nc()
    res = bass_utils.run_bass_kernel_spmd(nc, in_maps, core_ids=list(range(8)))
    outs = [res.results[c]["out"] for c in range(8)]
    full = np.stack([
        outs[0] + outs[1] + outs[2] + outs[3],
        outs[4] + outs[5] + outs[6] + outs[7],
    ]).astype(np.float32)
    return full


# revision 23
# speedup vs baseline: 1.2765x; 1.0556x over previous
"""DFlashAttention Trainium2 kernel (8 NeuronCores).

Sharding: batch (2) data-parallel x kv-head-group (4) tensor-parallel = 8 cores.
Core c handles batch b=c//4, kv head g=c%4, q heads [4g..4g+4).
o_proj partials are summed on host (the all-reduce).

Key structure (per core, software-pipelined over 9 kv blocks of 512):
 - K/V/Q projections in fp8 DoubleRow, 3 chains per projection at a uniform
   64x PSUM scale: wA@xhi + wA@xlo + rA@xhi with xhi=fp8(x), xlo=fp8(x-xhi),
   wA=fp8(64w), rA=fp8(64w-wA).  ~bf16-exact, 2.67x fewer PE cycles.
 - K/V projected TALL ([kv,hd]): rope is then a free-dim shuffle with a
   host-signed sin table (Pool), V needs no transpose at all, and the
   K rms term is a Pool square + DVE free-axis reduce -> msk[kv,1].
 - RMSNorm folded into softmax: probs = exp(st * scaleAP[kv]) where
   scaleAP = exp(-0.5*ln(mean(k~^2)+eps) + ln(SCALE/64)) via ACT Ln+Exp
   (both live in one activation table -> no table reloads; Sqrt banned).
   Q-side norm multiplies into qhat (PE broadcast); k_norm_w folded into
   q_norm_w on the host (elementwise product, both apply per head-dim).
 - Attention TALL: ot[q,hd] += P_slice^T(stationary) @ V[kv,hd], and the
   softmax denominators are stationary-P matmuls with a [128,1] output
   (~1 PE cycle each) accumulated across all blocks in a single PSUM bank
   via a zero-matmul start/stop bracket.  onecol=64.0 makes den=64*sum(p),
   cancelling the 64x V scale in the final reciprocal.
 - o_proj in bf16 as before; outputs DMA'd straight from PSUM.

dtypes: fp8e4(e4m3) for projection operands, bf16 scores/probs/attn/o_proj,
fp32 PSUM accumulation throughout, fp32 output.
"""

import numpy as np
import ml_dtypes

import concourse.bass as bass
import concourse.mybir as mybir
from concourse import bacc
from concourse.tile import TileContext
from concourse import bass_utils

F32 = mybir.dt.float32
F32R = mybir.dt.float32r
BF16 = mybir.dt.bfloat16
FP8 = mybir.dt.float8e4
DR = mybir.MatmulPerfMode.DoubleRow
AL = mybir.AluOpType
AF = mybir.ActivationFunctionType

B, CTX, DRAFT, D = 2, 4096, 512, 2048
H, KVH, HD = 16, 4, 128
NH = H // KVH            # 4 q heads per core
TOT = CTX + DRAFT        # 4608
BLK = 512
NB = TOT // BLK          # 9 kv blocks
SQ = DRAFT               # 512 queries
EPS = 1e-6
THETA = 10000.0
SCALE = 1.0 / float(np.sqrt(HD))
WS = 64.0                # fp8 weight pre-scale

_CACHE: dict = {}


def _build_nc(pend_depth: int = 4, x4_bufs: int = 16, pex_bufs: int = 10):
    nc = bacc.Bacc()

    xdh = nc.dram_tensor("xdh", [D, SQ], FP8, kind="ExternalInput")
    xdl = nc.dram_tensor("xdl", [D, SQ], FP8, kind="ExternalInput")
    xkh = nc.dram_tensor("xkh", [D, TOT], FP8, kind="ExternalInput")
    xkl = nc.dram_tensor("xkl", [D, TOT], FP8, kind="ExternalInput")
    wqa_d = nc.dram_tensor("wqa", [D, NH * HD], FP8, kind="ExternalInput")
    wqr_d = nc.dram_tensor("wqr", [D, NH * HD], FP8, kind="ExternalInput")
    wka_d = nc.dram_tensor("wka", [D, HD], FP8, kind="ExternalInput")
    wkr_d = nc.dram_tensor("wkr", [D, HD], FP8, kind="ExternalInput")
    wva_d = nc.dram_tensor("wva", [D, HD], FP8, kind="ExternalInput")
    wvr_d = nc.dram_tensor("wvr", [D, HD], FP8, kind="ExternalInput")
    wo_d = nc.dram_tensor("wo", [NH * HD, D], BF16, kind="ExternalInput")
    coskT_d = nc.dram_tensor("coskT", [TOT, HD], BF16, kind="ExternalInput")
    sinkT_d = nc.dram_tensor("sinkT", [TOT, HD], BF16, kind="ExternalInput")
    cosq_d = nc.dram_tensor("cosq", [HD, SQ], BF16, kind="ExternalInput")
    sinq_d = nc.dram_tensor("sinq", [HD, SQ], BF16, kind="ExternalInput")
    perm_d = nc.dram_tensor("perm", [HD, HD], F32R, kind="ExternalInput")
    identb_d = nc.dram_tensor("identb", [HD, HD], BF16, kind="ExternalInput")
    identr_d = nc.dram_tensor("identr", [HD, HD], F32R, kind="ExternalInput")
    onesc_d = nc.dram_tensor("onesc", [HD, 1], F32R, kind="ExternalInput")
    wqnk_d = nc.dram_tensor("wqnk", [1, HD], F32R, kind="ExternalInput")
    out = nc.dram_tensor("out", [SQ, D], F32, kind="ExternalOutput")

    with nc.allow_low_precision("fp8/f32r matmul pipeline"), \
         TileContext(nc) as tc:
        with (
            tc.tile_pool(name="const", bufs=1) as cpool,
            tc.tile_pool(name="wts", bufs=1) as wpool,
            tc.tile_pool(name="x4", bufs=x4_bufs) as x4pool,
            tc.tile_pool(name="ck", bufs=2) as ckpool,
            tc.tile_pool(name="scr", bufs=2) as scr,
            tc.tile_pool(name="pex", bufs=pex_bufs) as pex,
            tc.tile_pool(name="acc", bufs=1) as accp,
            tc.tile_pool(name="ps_proj", bufs=2, space="PSUM") as ps_proj,
            tc.tile_pool(name="ps_st", bufs=2, space="PSUM") as ps_st,
            tc.tile_pool(name="ps_tr", bufs=1, space="PSUM") as ps_tr,
            tc.tile_pool(name="ps_ot", bufs=2, space="PSUM") as ps_ot,
            tc.tile_pool(name="ps_den", bufs=1, space="PSUM") as ps_den,
        ):
            # ---- constants ----
            perm = cpool.tile([HD, HD], F32R, name="perm_sb")
            nc.scalar.dma_start(perm[:, :], perm_d[:, :])
            identb = cpool.tile([HD, HD], BF16, name="identb_sb")
            nc.scalar.dma_start(identb[:, :], identb_d[:, :])
            identr = cpool.tile([HD, HD], F32R, name="identr_sb")
            nc.scalar.dma_start(identr[:, :], identr_d[:, :])
            onesc = cpool.tile([HD, 1], F32R, name="onesc_sb")
            nc.scalar.dma_start(onesc[:, :], onesc_d[:, :])
            wqnk = cpool.tile([1, HD], F32R, name="wqnk_sb")
            nc.scalar.dma_start(wqnk[:, :], wqnk_d[:, :])
            cosq = cpool.tile([HD, SQ], BF16, name="cosq_sb")
            nc.scalar.dma_start(cosq[:, :], cosq_d[:, :])
            sinq = cpool.tile([HD, SQ], BF16, name="sinq_sb")
            nc.scalar.dma_start(sinq[:, :], sinq_d[:, :])
            eps_t = cpool.tile([128, 1], F32, name="eps_sb")
            nc.vector.memset(eps_t[:, :], EPS)
            zb = cpool.tile([128, 1], F32, name="zb_sb")
            nc.vector.memset(zb[:, :], 0.0)
            lnk_t = cpool.tile([128, 1], F32, name="lnk_sb")
            nc.vector.memset(lnk_t[:, :], float(np.log(SCALE / WS)))
            lnq_t = cpool.tile([1, 1], F32, name="lnq_sb")
            nc.vector.memset(lnq_t[:, :], float(np.log(1.0 / WS)))
            onecol64 = cpool.tile([128, 1], BF16, name="onecol64_sb")
            nc.vector.memset(onecol64[:, :], WS)
            zcolw = cpool.tile([128, HD], BF16, name="zcolw_sb")
            nc.vector.memset(zcolw[:, :], 0.0)
            zrow16 = cpool.tile([128, 16], BF16, name="zrow16_sb")
            nc.vector.memset(zrow16[:, :], 0.0)

            # ---- weights ----
            wqa = wpool.tile([128, 16, NH * HD], FP8, name="wqa_sb")
            nc.sync.dma_start(wqa[:, :, :],
                              wqa_d[:, :].rearrange("(j p) c -> p j c", p=128))
            wqr = wpool.tile([128, 16, NH * HD], FP8, name="wqr_sb")
            nc.sync.dma_start(wqr[:, :, :],
                              wqr_d[:, :].rearrange("(j p) c -> p j c", p=128))
            # xd split tiles (xdh on ACT queue, parallel with wqa on SP)
            xdh_sb = wpool.tile([128, 16, SQ], FP8, name="xdh_sb")
            nc.scalar.dma_start(xdh_sb[:, :, :],
                              xdh[:, :].rearrange("(j p) c -> p j c", p=128))
            xdl_sb = wpool.tile([128, 16, SQ], FP8, name="xdl_sb")
            nc.sync.dma_start(xdl_sb[:, :, :],
                              xdl[:, :].rearrange("(j p) c -> p j c", p=128))

            wk8 = {}
            for nm, dt_ in (("wka", wka_d), ("wkr", wkr_d),
                            ("wva", wva_d), ("wvr", wvr_d)):
                t = wpool.tile([128, 16, HD], FP8, name=f"{nm}_sb")
                nc.sync.dma_start(t[:, :, :],
                                  dt_[:, :].rearrange("(j p) h -> p j h", p=128))
                wk8[nm] = t

            # persistent accumulators
            otsbT = [accp.tile([128, NH, HD], F32, name=f"otsbT{h}")
                     for h in range(NH)]
            qrope = [accp.tile([HD, SQ], BF16, name=f"qrope{h}") for h in range(NH)]

            # ---- phase 1: Q projection (fp8 DR, 3 chains) + norm/rope ----
            # pass-major order: pass A (wqa@xhi) for all heads needs only the
            # first two DMAs; later passes consume wqr/xdl as they land.
            psqs = []
            for h in range(NH):
                pool_h = ps_st if h < 2 else ps_ot
                psqs.append(pool_h.tile([HD, SQ], F32, name=f"psq{h}",
                                        tag="st" if h < 2 else "ot"))
            for pi, (wtile, xtile) in enumerate(
                    ((wqa, xdh_sb), (wqr, xdh_sb), (wqa, xdl_sb))):
                for h in range(NH):
                    hs = slice(h * HD, (h + 1) * HD)
                    for t in range(8):
                        nc.tensor.matmul(psqs[h][:, :],
                                         wtile[:, 2 * t:2 * t + 2, hs],
                                         xtile[:, 2 * t:2 * t + 2, :],
                                         start=(pi == 0 and t == 0),
                                         stop=(pi == 2 and t == 7),
                                         perf_mode=DR)
            for h in range(NH):
                psq = psqs[h]
                # norm + rope chain (all ACT via Ln/Exp table)
                src = scr.tile([128, SQ], F32, name=f"qsrc{h}", tag="qsrc")
                nc.vector.tensor_copy(src[:, :], psq[:, :])
                sq = scr.tile([128, SQ], F32R, name=f"qsq{h}", tag="qsq")
                nc.gpsimd.tensor_mul(sq[:, :], src[:, :], src[:, :])
                ssq = ps_tr.tile([1, SQ], F32, name=f"qssq{h}", tag="tr")
                nc.tensor.matmul(ssq[:, :], onesc[:, :], sq[:, :],
                                 start=True, stop=True)
                lnm = scr.tile([1, SQ], F32, name=f"qln{h}", tag="qln")
                nc.scalar.activation(lnm[:, :], ssq[:, :], AF.Ln,
                                     bias=eps_t[0:1, :],
                                     scale=1.0 / (HD * WS * WS))
                rs = scr.tile([1, SQ], F32R, name=f"qrs{h}", tag="qln")
                nc.scalar.activation(rs[:, :], lnm[:, :], AF.Exp,
                                     bias=lnq_t[:, :], scale=-0.5)
                nf = ps_st.tile([128, SQ], F32, name=f"qnf{h}", tag="st")
                nc.tensor.matmul(nf[:, :], wqnk[:, :], rs[:, :],
                                 start=True, stop=True)
                xn = scr.tile([128, SQ], F32R, name=f"qxn{h}", tag="qxn")
                nc.vector.tensor_mul(xn[:, :], src[:, :], nf[:, :])
                pr = ps_st.tile([128, SQ], F32, name=f"qpr{h}", tag="st")
                nc.tensor.matmul(pr[:, :], perm[:, :], xn[:, :],
                                 start=True, stop=True)
                t1 = scr.tile([128, SQ], F32R, name=f"qt1{h}", tag="qsq")
                nc.gpsimd.tensor_mul(t1[:, :], xn[:, :], cosq[:, :])
                t2 = scr.tile([128, SQ], F32, name=f"qt2{h}", tag="qxn")
                nc.vector.tensor_mul(t2[:, :], pr[:, :], sinq[:, :])
                nc.gpsimd.tensor_add(qrope[h][:, :], t1[:, :], t2[:, :])

            # denominator accumulator bracket start (held across phase 2)
            den_ps = ps_den.tile([128, 16], F32, name="den_ps")
            nc.tensor.matmul(den_ps[:, :], zcolw[:, :], zrow16[:, :],
                             start=True, stop=False)

            # ---- phase 2: pipelined kv blocks ----
            state: dict = {}

            def load_block(cb):
                csl = slice(cb * BLK, (cb + 1) * BLK)
                xs = []
                for src_d, nm in ((xkh, "xh"), (xkl, "xl")):
                    dg_tiles = []
                    for dg in range(4):
                        t = x4pool.tile([128, 4, BLK], FP8,
                                        name=f"{nm}{cb}_{dg}", tag="x4")
                        nc.gpsimd.dma_start(
                            t[:, :, :],
                            src_d[dg * 512:(dg + 1) * 512, csl]
                            .rearrange("(j p) c -> p j c", p=128))
                        dg_tiles.append(t)
                    xs.append(dg_tiles)
                state[("x", cb)] = xs
                for src_d, nm in ((coskT_d, "cosk"), (sinkT_d, "sink")):
                    t = ckpool.tile([128, 4, HD], BF16, name=f"{nm}{cb}", tag=nm)
                    nc.sync.dma_start(
                        t[:, :, :],
                        src_d[csl, :].rearrange("(j p) h -> p j h", p=128))
                    state[(nm, cb)] = t

            def proj_block(cb):
                """K/V fp8-DR tall projections: out [kv,hd] per 128-chunk."""
                xh4, xl4 = state[("x", cb)]
                for wa, wr, key in (("wka", "wkr", "kt"), ("wva", "wvr", "vt")):
                    ps = ps_proj.tile([128, 4, HD], F32, name=f"{key}{cb}",
                                      tag="proj")
                    for c in range(4):
                        cs = slice(c * 128, (c + 1) * 128)
                        first = True
                        for wtile, x4 in ((wk8[wa], xh4), (wk8[wa], xl4),
                                          (wk8[wr], xh4)):
                            for dg in range(4):
                                for u in range(2):
                                    nc.tensor.matmul(
                                        ps[:, c, :],
                                        x4[dg][:, 2 * u:2 * u + 2, cs],
                                        wtile[:, dg * 4 + 2 * u:dg * 4 + 2 * u + 2, :],
                                        start=first,
                                        stop=(wtile is wk8[wr] and dg == 3
                                              and u == 1),
                                        perf_mode=DR)
                                    first = False
                    state[(key, cb)] = ps
                state.pop(("x", cb))

            def prep_block(cb):
                """rope K (Pool, signed-sin shuffle), msk->rsq, transpose K,
                copy V; produces ktf [hd,kv] bf16, vnat [kv,hd] bf16, rsq."""
                ktT = state.pop(("kt", cb))
                vtT = state.pop(("vt", cb))
                cosk = state.pop(("cosk", cb))
                sink = state.pop(("sink", cb))
                kts = scr.tile([128, 4, HD], F32R, name=f"kts{cb}", tag="kts")
                nc.vector.tensor_copy(kts[:, :, :], ktT[:, :, :])
                t1 = scr.tile([128, 4, HD], F32R, name=f"t1_{cb}", tag="t1")
                nc.gpsimd.tensor_mul(t1[:, :, :], kts[:, :, :], cosk[:, :, :])
                roped = scr.tile([128, 4, HD], F32R, name=f"rop{cb}", tag="rop")
                nc.gpsimd.tensor_mul(roped[:, :, 0:64], kts[:, :, 64:128],
                                     sink[:, :, 0:64])
                nc.gpsimd.tensor_mul(roped[:, :, 64:128], kts[:, :, 0:64],
                                     sink[:, :, 64:128])
                nc.gpsimd.tensor_add(roped[:, :, :], roped[:, :, :], t1[:, :, :])
                sq = scr.tile([128, 4, HD], F32R, name=f"sqk{cb}", tag="t1")
                nc.gpsimd.tensor_mul(sq[:, :, :], roped[:, :, :], roped[:, :, :])
                msk = scr.tile([128, 4], F32, name=f"msk{cb}", tag="msk")
                nc.vector.tensor_reduce(msk[:, :], sq[:, :, :],
                                        axis=mybir.AxisListType.X, op=AL.add)
                lnm = scr.tile([128, 4], F32, name=f"lnk{cb}", tag="msk")
                nc.scalar.activation(lnm[:, :], msk[:, :], AF.Ln,
                                     bias=eps_t[:, :],
                                     scale=1.0 / (HD * WS * WS))
                rsq = scr.tile([128, 4], F32, name=f"rsq{cb}", tag="rsq")
                nc.scalar.activation(rsq[:, :], lnm[:, :], AF.Exp,
                                     bias=lnk_t[:, :], scale=-0.5)
                # transpose roped K -> [hd, kv] (f32r), then one copy to bf16
                tr = ps_tr.tile([128, 4, HD], F32R, name=f"ktr{cb}", tag="tr")
                for j in range(4):
                    nc.tensor.transpose(tr[:, j, :], roped[:, j, :],
                                        identr[:, :])
                ktf = scr.tile([128, 4, HD], BF16, name=f"ktf{cb}", tag="ktf")
                nc.vector.tensor_copy(ktf[:, :, :], tr[:, :, :])
                vnat = scr.tile([128, 4, HD], BF16, name=f"vnat{cb}", tag="vnat")
                nc.vector.tensor_copy(vnat[:, :, :], vtT[:, :, :])
                state[("ktf", cb)] = ktf
                state[("vnat", cb)] = vnat
                state[("rsq", cb)] = rsq

            rd = accp.tile([128, 16], F32, name="rd_sb")
            otn = [None] * NH

            def normalize_head(h):
                """per-head: rd slice, normalize (ACT Copy w/ scale),
                transpose back to [hd, q].  Fired from the last attn block
                so o_proj isn't gated on a serial phase-3 chain."""
                cs = slice(4 * h, 4 * h + 4)
                nc.vector.reciprocal(rd[:, cs], den_ps[:, cs])
                otnT = accp.tile([128, NH, HD], BF16, name=f"otnT{h}")
                for qc in range(4):
                    nc.scalar.activation(otnT[:, qc, :], otsbT[h][:, qc, :],
                                         AF.Copy,
                                         scale=rd[:, 4 * h + qc:4 * h + qc + 1])
                trh = ps_tr.tile([128, 4, HD], BF16, name=f"otr{h}", tag="tr")
                for qc in range(4):
                    nc.tensor.transpose(trh[:, qc, :], otnT[:, qc, :],
                                        identb[:, :])
                o_h = accp.tile([128, NH, HD], BF16, name=f"otn{h}")
                nc.vector.tensor_copy(o_h[:, :, :], trh[:, :, :])
                otn[h] = o_h

            def attn_block(cb):
                ktf = state.pop(("ktf", cb))
                vnat = state.pop(("vnat", cb))
                rsq = state.pop(("rsq", cb))
                pend = []

                def flush_one():
                    h, j, p_t, ot_ps = pend.pop(0)
                    for qc in range(4):
                        qs = slice(qc * 128, (qc + 1) * 128)
                        nc.tensor.matmul(ot_ps[:, qc, :], p_t[:, qs],
                                         vnat[:, j, :],
                                         start=(j == 0 and qc == 0),
                                         stop=(j == 3 and qc == 3))
                        nc.tensor.matmul(den_ps[:, 4 * h + qc:4 * h + qc + 1],
                                         p_t[:, qs], onecol64[:, :],
                                         start=False, stop=False)
                    if j == 3:
                        if cb == 0:
                            nc.vector.tensor_copy(otsbT[h][:, :, :],
                                                  ot_ps[:, :, :])
                        else:
                            nc.vector.tensor_add(otsbT[h][:, :, :],
                                                 otsbT[h][:, :, :],
                                                 ot_ps[:, :, :])
                        if cb == NB - 1:
                            normalize_head(h)

                for h in range(NH):
                    ot_ps = ps_ot.tile([128, 4, HD], F32, name=f"ot{cb}_{h}",
                                       tag="ot")
                    for j in range(4):
                        st_ps = ps_st.tile([128, SQ], F32,
                                           name=f"st{cb}_{h}_{j}", tag="st")
                        nc.tensor.matmul(st_ps[:, :], ktf[:, j, :],
                                         qrope[h][:, :], start=True, stop=True)
                        p_t = pex.tile([128, SQ], BF16, name=f"p{cb}_{h}_{j}",
                                       tag="pex")
                        nc.scalar.activation(p_t[:, :], st_ps[:, :], AF.Exp,
                                             bias=zb[:, :],
                                             scale=rsq[:, j:j + 1])
                        pend.append((h, j, p_t, ot_ps))
                        if len(pend) >= pend_depth:
                            flush_one()
                while pend:
                    flush_one()

            # pipeline
            load_block(0)
            load_block(1)
            proj_block(0)
            prep_block(0)
            for cb in range(NB):
                if cb + 2 < NB:
                    load_block(cb + 2)
                if cb + 1 < NB:
                    proj_block(cb + 1)
                attn_block(cb)
                if cb + 1 < NB:
                    prep_block(cb + 1)
                if cb == NB - 2:
                    woN = []
                    for n in range(4):
                        t = x4pool.tile([128, 4, 512], BF16, name=f"woN{n}",
                                        tag="x4")
                        nc.sync.dma_start(
                            t[:, :, :],
                            wo_d[:, n * 512:(n + 1) * 512]
                            .rearrange("(h p) c -> p h c", p=128))
                        woN.append(t)

            # ---- phase 3: close denominator group, o_proj ----
            nc.tensor.matmul(den_ps[:, :], zcolw[:, :], zrow16[:, :],
                             start=False, stop=True)
            for n in range(4):
                for m in range(4):
                    po = ps_st.tile([128, 512], F32, name=f"po{n}_{m}", tag="st")
                    for h in range(NH):
                        nc.tensor.matmul(po[:, :], otn[h][:, m, :],
                                         woN[n][:, h, :],
                                         start=(h == 0), stop=(h == 3))
                    osb = scr.tile([128, 512], F32, name=f"osb{n}_{m}",
                                   tag="osb", bufs=3)
                    if (n + m) % 2 == 0:
                        nc.scalar.activation(osb[:, :], po[:, :], AF.Copy)
                    else:
                        nc.vector.tensor_copy(osb[:, :], po[:, :])
                    nc.sync.dma_start(
                        out[m * 128:(m + 1) * 128, n * 512:(n + 1) * 512],
                        osb[:, :])
    # All activation funcs used (Exp, Ln, Copy) live in one table set
    # (natural_log_exp_and_others).  The default greedy pass picks a
    # different "first matching" set per function and thrashes 26 table
    # reloads (~33us on ACT); pin the single covering set instead.
    import types
    from concourse.hw_specs import get_activation_tables

    def _pin_act_table(self):
        tables = list(get_activation_tables(self.m.arch).items())
        idx = [i for i, (nm, fs) in enumerate(tables)
               if nm == "natural_log_exp_and_others"][0]
        funcs = tables[idx][1]
        for blk in self.main_func.blocks:
            pos = None
            for i, inst in enumerate(blk.instructions):
                if isinstance(inst, mybir.InstActivation):
                    assert inst.func in funcs, f"{inst.func} not in pinned set"
                    if pos is None:
                        pos = i
            if pos is None:
                continue
            atl = mybir.InstLoadActFuncSet(
                name=self.get_next_instruction_name(), ins=[], outs=[],
                act_func_set_id=idx)
            atl.engine = mybir.EngineType.Activation
            self.register_instruction(atl)
            blk.instructions.insert(pos, atl)

    nc.insert_act_table_loads = types.MethodType(_pin_act_table, nc)
    nc.finalize()
    return nc


def get_nc(**kw):
    key = ("nc", tuple(sorted(kw.items())))
    if key not in _CACHE:
        _CACHE[key] = _build_nc(**kw)
    return _CACHE[key]


def _host_tables():
    inv = 1.0 / (THETA ** (np.arange(0, HD, 2, dtype=np.float32) / np.float32(HD)))
    inv2 = np.concatenate([inv, inv]).astype(np.float32)  # [128]
    pm = np.zeros((HD, HD), np.float32)
    pm[np.arange(64) + 64, np.arange(64)] = -1.0
    pm[np.arange(64), np.arange(64) + 64] = 1.0
    ident = np.eye(HD, dtype=np.float32)
    onesc = np.ones((HD, 1), np.float32)
    return inv2, pm, ident, onesc


def _make_in_maps(inputs):
    F8 = ml_dtypes.float8_e4m3
    bf = ml_dtypes.bfloat16
    draft = np.ascontiguousarray(np.asarray(inputs["draft_hidden"], np.float32))
    ctx = np.ascontiguousarray(np.asarray(inputs["context_hidden"], np.float32))
    Wq = np.asarray(inputs["Wq"], np.float32)
    Wk = np.asarray(inputs["Wk"], np.float32)
    Wv = np.asarray(inputs["Wv"], np.float32)
    Wo = np.asarray(inputs["Wo"], np.float32)
    qnw = np.asarray(inputs["q_norm_w"], np.float32).reshape(HD)
    knw = np.asarray(inputs["k_norm_w"], np.float32).reshape(HD)
    cpos = np.asarray(inputs["context_position_ids"])
    dpos = np.asarray(inputs["draft_position_ids"])

    inv2, pm, ident, onesc = _host_tables()

    def split8(x):
        hi = x.astype(F8)
        lo = (x - hi.astype(np.float32)).astype(F8)
        return hi, lo

    def wsplit(w):
        a = (WS * w).astype(F8)
        r = (WS * w - a.astype(np.float32)).astype(F8)
        return a, r

    in_maps = []
    for c in range(8):
        b, g = c // 4, c % 4
        kvin = np.concatenate([ctx[b], draft[b]], axis=0)        # [4608, 2048]
        xkvT = np.ascontiguousarray(kvin.T)                      # [2048, 4608]
        xdT = np.ascontiguousarray(draft[b].T)                   # [2048, 512]
        xkh, xkl = split8(xkvT)
        xdh, xdl = split8(xdT)
        wqa, wqr = wsplit(np.ascontiguousarray(
            Wq[4 * g * HD:(4 * g + 4) * HD, :].T))               # [2048, 512]
        wka, wkr = wsplit(np.ascontiguousarray(Wk[g * HD:(g + 1) * HD, :].T))
        wva, wvr = wsplit(np.ascontiguousarray(Wv[g * HD:(g + 1) * HD, :].T))
        woT = np.ascontiguousarray(Wo[:, 4 * g * HD:(4 * g + 4) * HD].T)

        fpos = np.concatenate([cpos[b], dpos[b]]).astype(np.float32)  # [4608]
        angkT = fpos[:, None] * inv2[None, :]                     # [4608, 128]
        coskT = np.cos(angkT)
        sinkT = np.sin(angkT)
        sinkT[:, 0:64] = -sinkT[:, 0:64]                          # host-signed
        angq = inv2[:, None] * dpos[b].astype(np.float32)[None, :]  # [128, 512]

        in_maps.append({
            "xdh": xdh, "xdl": xdl, "xkh": xkh, "xkl": xkl,
            "wqa": wqa, "wqr": wqr, "wka": wka, "wkr": wkr,
            "wva": wva, "wvr": wvr,
            "wo": woT.astype(bf),
            "coskT": coskT.astype(bf), "sinkT": sinkT.astype(bf),
            "cosq": np.cos(angq).astype(bf), "sinq": np.sin(angq).astype(bf),
            "perm": pm, "identb": ident.astype(bf), "identr": ident,
            "onesc": onesc,
            "wqnk": (qnw * knw).reshape(1, HD),
        })
    return in_maps


def kernel(**inputs):
    in_maps = _make_in_maps(inputs)
    nc = get_nc()
    res = bass_utils.run_bass_kernel_spmd(nc, in_maps, core_ids=list(range(8)))
    outs = [res.results[c]["out"] for c in range(8)]
    full = np.stack([
        outs[0] + outs[1] + outs[2] + outs[3],
        outs[4] + outs[5] + outs[6] + outs[7],
    ]).astype(np.float32)
    return full
